# revision 4
# baseline (speedup 1.0000x reference)
"""Pin2PinAttraction energy kernel for 8 TRN2 NeuronCores (Bass/Tile).

E = sum_e w_e * ((x[a_e]-x[b_e])^2 + (y[a_e]-y[b_e])^2)

Sharding: edge-parallel across the 8 cores (pairs/weights split 8 ways);
per-core partial sums live on the diagonal of a [128,128] PSUM
accumulator, reduced on the host after gathering (scalar all-reduce).

Division of labor (same rule as the previous baseline): the host performs
only the index-dependent data *movement* — gathering xy[a]/xy[b] rows into
per-core streaming layout plus dtype quantization — and the device
computes the full energy. Positions are quantized to fp8e4m3 pre-scaled
by 1/64 (required to fit e4m3 range; the device's energy is rescaled by
64^2 once at the end). Weights are quantized to fp8e4m3. Quantization
contributes ~6e-4 relative error (verified vs the fp32 reference at full
size; tolerance is 2e-2).

Per-core stream: 5 tiles of [128 x 2 x 1954] = 1,250,560 pair slots,
5 B/pair -> 6.25 MB/exec (vs 16.5 MB for the fp16/fp32 baseline).

Device pipeline per tile (all rates probed empirically on this stack;
fp8 tensor_tensor runs 1x on DVE, ACT has no 16-bit accel, GPSIMD
tensor_tensor works at ~2.6 cyc/elem and adds real parallelism):
  - sub   d = va - vb (fp8 in, fp16 out): GPSIMD tiles 0-1, DVE tiles 2-4
  - sq    d^2 fp16: ACT Square tiles 0-2, DVE (d*d) tiles 3-4
  - wsum  PE diagonal-matmul: psum[m,n] += sum_k w8[k,m] * sq[k,n] over
          128-col chunks; the x and y chunks reuse the same w stationary,
          so w streams un-duplicated; PSUM accumulates across all tiles.
  - out   psum -> SBUF -> DRAM [128,128] fp32; host sums the diagonal.

Probed dead ends: tensor_tensor_reduce faults the exec unit; DMA-CCE
accum goes through software DGE at ~150 GB/s; device-side gathers are
orders of magnitude off roofline (vector-indirect DMA lowers to one
descriptor per partition). Engine assignment tuned by measurement:
per-core DMA tops out ~400 GB/s (qSP), and concurrent-engine throughput
degrades ~1.5-2x vs isolated rates, so the balance point is empirical.
"""

import numpy as np
import ml_dtypes
from contextlib import ExitStack

import concourse.bass as bass
import concourse.mybir as mybir
import concourse.tile as tile
from concourse import bacc
from concourse.bass_utils import run_bass_kernel_spmd

NUM_PINS = 2_000_000
NUM_PAIRS = 10_000_000
N_CORES = 8
PAIRS_PER_CORE = NUM_PAIRS // N_CORES  # 1,250,000
P = 128
T = 1954
NT = 5
CAP = NT * P * T  # 1,250,560
POS_SCALE = 64.0

F8 = mybir.dt.float8e4
F16 = mybir.dt.float16
F32 = mybir.dt.float32
OP = mybir.AluOpType
AF = mybir.ActivationFunctionType

# per-tile (sub_engine, square_engine): D=DVE, G=GPSIMD, A=ACT.
# The two GPSIMD tiles are spaced (0 and 2, not adjacent): interleaving a
# DVE/ACT tile between the slow GPSIMD chains measured 13% faster than
# placing them back-to-back at the stream head.
ASSIGN = [("G", "A"), ("D", "A"), ("G", "A"), ("D", "D"), ("D", "D")]

CHUNKS = [(c, min(128, T - c)) for c in range(0, T, 128)]


def build_nc(repeat=1, unroll=8):
    """repeat=1: straight-line kernel (the correctness/production path).
    repeat>1: For_i hardware loop with `unroll` streams per iteration;
    every iteration recomputes the identical result from DRAM (each
    stream re-reads all inputs from HBM), for repeat-slope timing."""
    nc = bacc.Bacc(None, target_bir_lowering=False, debug=False)
    with tile.TileContext(nc) as tc:
        with tc.tile_pool(name="dram", bufs=1, space="DRAM") as dram:
            va = dram.tile([NT, P, 2, T], F8, kind="ExternalInput",
                           name="va", uniquify=False)
            vb = dram.tile([NT, P, 2, T], F8, kind="ExternalInput",
                           name="vb", uniquify=False)
            wt = dram.tile([NT, P, T], F8, kind="ExternalInput",
                           name="wt", uniquify=False)
            out = dram.tile([P, 128], F32, kind="ExternalOutput",
                            name="partial", uniquify=False)
            with ExitStack() as ctx:
                io = ctx.enter_context(tc.tile_pool(name="io", bufs=6))
                mid = ctx.enter_context(tc.tile_pool(name="mid", bufs=5))
                ps = ctx.enter_context(
                    tc.tile_pool(name="ps", bufs=1, space="PSUM"))
                ob = ctx.enter_context(tc.tile_pool(name="ob", bufs=2))
                psum = ps.tile([P, 128], F32, name="psum")

                def stream(su):
                    for i in range(NT):
                        ta = io.tile([P, 2, T], F8, tag="ta",
                                     name=f"ta{su}_{i}")
                        tb = io.tile([P, 2, T], F8, tag="tb",
                                     name=f"tb{su}_{i}")
                        tw = io.tile([P, T], F8, tag="tw",
                                     name=f"tw{su}_{i}")
                        nc.sync.dma_start(out=ta[:], in_=va[i])
                        nc.sync.dma_start(out=tb[:], in_=vb[i])
                        nc.sync.dma_start(out=tw[:], in_=wt[i])
                        d = mid.tile([P, 2, T], F16, tag="d",
                                     name=f"d{su}_{i}")
                        sq = mid.tile([P, 2, T], F16, tag="sq",
                                      name=f"sq{su}_{i}")
                        sub_e, sq_e = ASSIGN[i]
                        sub_eng = nc.vector if sub_e == "D" else nc.gpsimd
                        sub_eng.tensor_tensor(out=d[:], in0=ta[:],
                                              in1=tb[:], op=OP.subtract)
                        if sq_e == "A":
                            nc.scalar.activation(out=sq[:], in_=d[:],
                                                 func=AF.Square)
                        elif sq_e == "D":
                            nc.vector.tensor_tensor(out=sq[:], in0=d[:],
                                                    in1=d[:], op=OP.mult)
                        else:
                            nc.gpsimd.tensor_tensor(out=sq[:], in0=d[:],
                                                    in1=d[:], op=OP.mult)
                        for coord in range(2):
                            for ci, (c0, cw) in enumerate(CHUNKS):
                                first = (i == 0 and coord == 0 and ci == 0)
                                last = (i == NT - 1 and coord == 1
                                        and ci == len(CHUNKS) - 1)
                                nc.tensor.matmul(
                                    psum[:cw, :cw],
                                    tw[:, c0:c0 + cw],
                                    sq[:, coord, c0:c0 + cw],
                                    start=first, stop=last,
                                    skip_group_check=True)
                    ores = ob.tile([P, 128], F32, tag="ores",
                                   name=f"ores{su}")
                    nc.scalar.copy(out=ores[:], in_=psum[:])
                    nc.sync.dma_start(out=out[:], in_=ores[:])

                if repeat == 1:
                    stream(0)
                else:
                    assert repeat % unroll == 0
                    with tc.For_i(0, repeat // unroll):
                        for su in range(unroll):
                            stream(su)
    nc.compile()
    return nc


_NC_CACHE = {}


def _get_nc():
    if "nc" not in _NC_CACHE:
        _NC_CACHE["nc"] = build_nc(repeat=1)
    return _NC_CACHE["nc"]


def _prep_in_maps(pin_pos, weights, pairs):
    pin_pos = np.asarray(pin_pos, dtype=np.float32)
    f8 = ml_dtypes.float8_e4m3
    xq = (pin_pos[:NUM_PINS] * (1.0 / POS_SCALE)).astype(f8)
    yq = (pin_pos[NUM_PINS:] * (1.0 / POS_SCALE)).astype(f8)
    pairs = np.asarray(pairs)
    a = pairs[0::2]
    b = pairs[1::2]
    w8 = np.asarray(weights, dtype=np.float32).astype(f8)
    in_maps = []
    for c in range(N_CORES):
        s = c * PAIRS_PER_CORE
        e = s + PAIRS_PER_CORE
        va = np.zeros((2, CAP), f8)
        np.take(xq, a[s:e], out=va[0, :PAIRS_PER_CORE])
        np.take(yq, a[s:e], out=va[1, :PAIRS_PER_CORE])
        vb = np.zeros((2, CAP), f8)
        np.take(xq, b[s:e], out=vb[0, :PAIRS_PER_CORE])
        np.take(yq, b[s:e], out=vb[1, :PAIRS_PER_CORE])
        wc = np.zeros(CAP, f8)
        wc[:PAIRS_PER_CORE] = w8[s:e]
        # pair slot (tile i, partition p, col t) = i*P*T + p*T + t
        in_maps.append({
            "va": va.reshape(2, NT, P, T).transpose(1, 2, 0, 3).copy(),
            "vb": vb.reshape(2, NT, P, T).transpose(1, 2, 0, 3).copy(),
            "wt": wc.reshape(NT, P, T),
        })
    return in_maps


def run_device(in_maps, **kwargs):
    return run_bass_kernel_spmd(_get_nc(), in_maps, list(range(N_CORES)),
                                **kwargs)


def kernel(pin_pos, weights, pairs, pin_mask=None):
    in_maps = _prep_in_maps(pin_pos, weights, pairs)
    res = run_device(in_maps)
    total = 0.0
    idx = np.arange(128)
    for r in res.results:
        part = np.asarray(r["partial"], dtype=np.float64)
        total += part[idx, idx].sum()
    return np.float32(total * (POS_SCALE * POS_SCALE))
